# revision 45
# baseline (speedup 1.0000x reference)
"""Trainium2 Bass kernel for masked cosine-similarity attention scores.

Problem: nn_MultiHeadedAttention_2 (sparse_attention, memory-bound)
  query [16, 1, 1024] f32, key [16, 8192, 1024] f32, mask [16, 8192] int32
  out   [16, 16, 8192] f32 = relu(cos_sim_per_head(q, k) masked) / Lk

Math (per batch b, head h, key position l):
  num[h,l] = sum_d q[h,d] * k[l, h*64+d]
  kn[h,l]  = ||k[l, h*64:(h+1)*64]||
  p        = relu(num / (qn[h] * kn)) * mask[l] / Lk
           = relu(sum_d qtilde[h,d] * k[...]) * exp(-0.5*ln(kn^2) + lnm[h,l])
  where qtilde = q / (qn * Lk) is folded on the host (input prep) and
  lnm = 0 if mask else -1e30 (exp(...-1e30) == 0 -> exact masked zero).
  The reference's EPS=1e-8 guard on qn*kn is unreachable for randn inputs.

Sharding: data-parallel over batch B=16 -> 2 batches per core x 8 cores.

Engine layout (from NTFF hardware profiles; v1 of this kernel ran 434us/core,
this version ~315us/core):
  * All elementwise/reduce work is grouped 8 key-subtiles (1024 keys) per
    instruction: [128, 8192]-element ops, so the ~60ns DVE issue overhead and
    ~400ns small-op costs stop dominating (v1 had 876 small DVE ops).
  * ALL TT/reduce work runs on DVE: GPSIMD shares DVE's SBUF ports, so
    splitting folds onto it just splits the same bandwidth (measured: both
    engines slow ~1.4x when run concurrently). GPSIMD only runs the casting
    DMA descriptors. DVE tensor_tensor runs in 2x bf16 mode (4426ns per
    [128,8192] op, matching the (N/2+151)/0.96GHz formula).
  * num path: TT mult + fold chain 64->4 (2x mode) + one 1x tensor_reduce.
    tensor_reduce is 1x-only on TRN2, so folds do most of the reduction.
  * k^2 path: ACT Square (scalar engine, own port) + same DVE fold chain.
  * mask folded via lnm16 (per-(t,h) -1e30 offsets) + STT before a grouped
    Exp (ACT's bias operand is per-partition only, so exp(scale*ln+bias)
    cannot be grouped directly).
  * key streaming split across two DMA paths in parallel (alternating
    groups): SWDGE f32->bf16 cast (qPoolDynamic, ~250GB/s read) and HWDGE
    f32 (qSyncDynamicHW) + ACT cast, exceeding the single-queue rate.
  * outputs transposed head-major via TensorE (idle otherwise) in [128,128]
    blocks, drained by ACT, one strided HWDGE store per (batch, tm).

Self-contained: only imports the platform libs from /opt/trn_rl_repo.
"""

import sys

sys.path.insert(0, "/opt/trn_rl_repo")

import numpy as np

import concourse.bass as bass
import concourse.mybir as mybir
from concourse.tile import TileContext

# Keep the number of active DMA completion-sem lanes low: the kernel-tail
# Drain waits on every active proc's semaphore and walrus rejects
# instructions with too many sync waits.
import concourse.tile_sem_assignment as _tsa

_tsa.NUM_HWDGE_SEMS = 2
_tsa.NUM_SWDGE_GLOBAL_SEMS = 2

# The walrus build in this environment accepts at most ONE sync wait per
# instruction. Tile's scheduler can emit several (cross-engine RAW + WAR +
# DMA-lane waits). Splitting the extra waits into standalone EventSemaphore
# instructions on the same engine is semantically identical: the engine's
# sequencer executes them in order immediately before the instruction.
import orjson as _orjson


def _split_multi_waits(bir_bytes: bytes) -> bytes:
    m = _orjson.loads(bir_bytes)
    changed = False
    for fn in m.get("functions", []):
        for bb in fn.get("blocks", []):
            insts = bb.get("instructions")
            if not insts:
                continue
            out_list = []
            for inst in insts:
                si = inst.get("sync_info")
                waits = (si or {}).get("on_wait") or []
                if len(waits) > 1:
                    changed = True
                    for k, w in enumerate(waits[:-1]):
                        out_list.append(
                            {
                                "debug": inst.get("debug", 0),
                                "engine": inst["engine"],
                                "ins": [],
                                "name": f"{inst['name']}_wsplit{k}",
                                "opcode": "EventSemaphore",
                                "outs": [],
                                "sync_info": {"on_update": [], "on_wait": [w]},
                            }
                        )
                    si["on_wait"] = [waits[-1]]
                out_list.append(inst)
            bb["instructions"] = out_list
    return _orjson.dumps(m) if changed else bir_bytes


_orig_to_json_bytes = bass.Bass.to_json_bytes


def _patched_to_json_bytes(self, *a, **kw):
    return _split_multi_waits(_orig_to_json_bytes(self, *a, **kw))


bass.Bass.to_json_bytes = _patched_to_json_bytes

F32 = mybir.dt.float32
BF16 = mybir.dt.bfloat16
I32 = mybir.dt.int32
Alu = mybir.AluOpType
Act = mybir.ActivationFunctionType
AX = mybir.AxisListType

H = 16      # heads
DK = 64     # head dim
DM = 1024   # d_model
P = 128     # SBUF partitions
N_CORES = 8
TG = 8      # 128-key subtiles per group
# Each group's batch 0 loads via SWDGE f32->bf16 cast (qPoolDynamic) and
# batch 1 via HWDGE f32 (qSyncDynamicHW) + ACT cast, so BOTH DMA queues
# stream on every group instead of alternating between groups.


def self_fold_reduce(nc, pool, src, out, tag, d0=DK):
    """Segmented sum over d0-wide segments of src [P, S*d0] (bf16)
    -> out [P, S] f32, via DVE 2x-mode fold chain down to 4 + one 1x reduce.
    All instructions on DVE (GPSIMD shares DVE's SBUF ports; using it
    concurrently just splits the same bandwidth)."""
    S = src.free_size() // d0
    cur, d = src, d0
    while d > 4:
        nxt = pool.tile([P, S * d // 2], src.dtype, name=f"fold{tag}{d}",
                        tag=f"fold{d // 2}")
        c3 = cur.rearrange("p (s d) -> p s d", d=d)
        nc.vector.tensor_tensor(
            nxt.rearrange("p (s d) -> p s d", d=d // 2),
            c3[:, :, 0 : d // 2],
            c3[:, :, d // 2 : d],
            Alu.add,
        )
        cur, d = nxt, d // 2
    nc.vector.reduce_sum(out[:], cur.rearrange("p (s d) -> p s d", d=d),
                         axis=AX.X)


def build_nc(n_batch: int, lk: int) -> bass.Bass:
    """Per-core Bass program.

    Per-core DRAM I/O:
      key   [n_batch, lk, 1024] f32   (shard of the key tensor)
      qb    [n_batch, 128, 1024] bf16 (host-broadcast qtilde rows)
      maskr [n_batch, 128, lk/128] i32 (mask with l split as l = t*128 + p)
      out   [n_batch, 16, lk] f32
    """
    assert n_batch == 2, "kernel assumes a batch pair per core"
    cdt = BF16
    ntiles = lk // P
    ngroups = ntiles // TG

    nc = bass.Bass()
    key_in = nc.declare_dram_parameter("key", [n_batch, lk, DM], F32, isOutput=False)
    qb_in = nc.declare_dram_parameter("qb", [n_batch, P, DM], cdt, isOutput=False)
    mask_in = nc.declare_dram_parameter(
        "maskr", [n_batch, P, ntiles], I32, isOutput=False
    )
    ident_in = nc.declare_dram_parameter("ident", [P, P], F32, isOutput=False)
    out = nc.declare_dram_parameter("out", [n_batch, H, lk], F32, isOutput=True)
    # outacc partition = b*64 + tm*16 + h; key position l = (gq*4 + tm)*128 + k
    out_r = out.rearrange("b h (gq tm k) -> b h gq tm k", tm=4, k=P)

    with TileContext(nc) as tc:
        with (
            tc.tile_pool(name="const", bufs=1) as cpool,
            tc.tile_pool(name="kbig", bufs=3) as kpool,
            tc.tile_pool(name="stage", bufs=3) as stpool,
            tc.tile_pool(name="psboth", bufs=2) as pbpool,
            tc.tile_pool(name="halfp", bufs=1) as hpool,
            tc.tile_pool(name="small", bufs=4) as spool,
            tc.tile_pool(name="ppairp", bufs=3) as pppool,
            tc.tile_pool(name="outp", bufs=1) as opool,
            tc.tile_pool(name="psum", bufs=4, space="PSUM") as pspool,
        ):
            def load_group(t0g, TGg):
                kts = []
                for b in range(n_batch):
                    kt = kpool.tile([P, TGg * DM], cdt, name="kt", tag="kt")
                    src = key_in[b].rearrange("(t p) c -> p t c", p=P)[
                        :, t0g : t0g + TGg, :
                    ]
                    dst = kt.rearrange("p (t c) -> p t c", c=DM)
                    if b == 0:
                        # SWDGE cast f32->bf16
                        nc.gpsimd.dma_start(out=dst, in_=src)
                    else:
                        # HWDGE f32 (four quarters) + ACT cast
                        QTR = TGg * DM // 4
                        for hf in range(4):
                            st = stpool.tile([P, QTR], F32, name="st", tag="st")
                            nc.sync.dma_start(
                                out=st.rearrange("p (t c) -> p t c", c=DM),
                                in_=src[
                                    :, hf * (TGg // 4) : (hf + 1) * (TGg // 4), :
                                ],
                            )
                            nc.scalar.copy(kt[:, hf * QTR : (hf + 1) * QTR], st[:])
                    kts.append(kt)
                return kts

            sizes = [TG] * (ntiles // TG)
            assert sum(sizes) == ntiles
            # prefetch group 0's key tiles BEFORE the constant loads so the
            # DMA queues start on the critical 4MB immediately (the constants
            # otherwise sit ahead of it in the queue FIFOs: ~19us of ramp)
            kts0 = load_group(0, sizes[0])

            # ---- constants / setup ----
            ident_r = cpool.tile([P, P], F32, name="ident_r")
            nc.gpsimd.dma_start(out=ident_r[:], in_=ident_in[:])
            ident = cpool.tile([P, P], F32, name="ident_s")
            nc.vector.tensor_copy(ident[:], ident_r[:])

            qbs, lnm16s = [], []
            for b in range(n_batch):
                qb_r = cpool.tile([P, DM], cdt, name=f"qbr{b}")
                nc.gpsimd.dma_start(out=qb_r[:], in_=qb_in[b])
                qb_s = cpool.tile([P, DM], cdt, name=f"qbs{b}")
                nc.scalar.copy(qb_s[:], qb_r[:])
                qbs.append(qb_s)

                maskt = cpool.tile([P, ntiles], I32, name=f"maskt{b}")
                nc.gpsimd.dma_start(out=maskt[:], in_=mask_in[b])
                maskf = cpool.tile([P, ntiles], F32, name=f"maskf{b}")
                nc.vector.tensor_copy(maskf[:], maskt[:])
                # expand [P, t] -> [P, t, 16] (repeat per head) in place,
                # then lnm16 = (m16 - 1) * 1e30  ->  {0 -> -1e30, 1 -> 0}
                lnm16 = cpool.tile([P, ntiles * H], F32, name=f"lnm16_{b}")
                m3 = lnm16.rearrange("p (t h) -> p t h", h=H)
                nc.vector.tensor_copy(
                    m3[:, :, 0:1], maskf.rearrange("p (t o) -> p t o", o=1)
                )
                w = 1
                while w < H:
                    nc.vector.tensor_copy(m3[:, :, w : 2 * w], m3[:, :, 0:w])
                    w *= 2
                # lnm2 = (m - 1) * -2e30 -> {0 -> +2e30, 1 -> 0}; Exp's
                # scale=-0.5 turns +2e30 into -1e30 -> exp -> exact 0
                nc.vector.tensor_scalar(
                    lnm16[:], lnm16[:], -1.0, -2.0e30, Alu.add, Alu.mult
                )
                lnm16s.append(lnm16)

            outacc = opool.tile([P, ngroups * 2 * P], F32, name="outacc")

            t0g = 0
            gq = 0  # global output quad index
            for gi, TGg in enumerate(sizes):
                kts = kts0 if gi == 0 else load_group(t0g, TGg)

                nquad = TGg // 4
                TH_g = TGg * H
                ppair = pppool.tile([P, nquad * P], F32, name="ppair", tag="ppair")
                for b in range(n_batch):
                    kt = kts[b]
                    # merged num + k^2 pipeline: products in the left half,
                    # squares in the right half of ONE double-buffered tile,
                    # so a single fold chain + reduce covers both segmented
                    # sums (4 fewer DVE issues/gb). Double-buffering (funded
                    # by the stride-0 qb broadcast replacing the 16KB qb8
                    # tiles) avoids the cross-group serialization that sank
                    # the single-buffered version of this merge.
                    ps = pbpool.tile([P, 2 * TGg * DM], cdt, name="ps",
                                     tag="ps")
                    qv = qbs[b][:]
                    qbc = bass.AP(
                        qv.tensor, qv.offset, [qv.ap[0], [0, TGg], qv.ap[1]]
                    )
                    nc.vector.tensor_tensor(
                        ps[:, 0 : TGg * DM].rearrange("p (t c) -> p t c", c=DM),
                        kt.rearrange("p (t c) -> p t c", c=DM),
                        qbc,
                        Alu.mult,
                    )
                    nc.scalar.activation(
                        ps[:, TGg * DM : 2 * TGg * DM], kt[:], Act.Square
                    )
                    nsb = spool.tile([P, 2 * TH_g], F32, name="nsb", tag="nsb")
                    self_fold_reduce(nc, hpool, ps, nsb, "b")
                    ns_num = nsb[:, 0:TH_g]
                    ns_sq = nsb[:, TH_g : 2 * TH_g]
                    # rk = exp(-0.5*(ln(s2) + lnm2)); the tiny add runs on
                    # GPSIMD (idle; [P,128] is too small to contend with DVE)
                    lns = spool.tile([P, TH_g], F32, name="lns", tag="lns")
                    nc.scalar.activation(lns[:], ns_sq[:], Act.Ln)
                    z = spool.tile([P, TH_g], F32, name="z", tag="z")
                    nc.gpsimd.tensor_tensor(
                        z[:],
                        lns[:],
                        lnm16s[b][:, t0g * H : (t0g + TGg) * H],
                        Alu.add,
                    )
                    rk = spool.tile([P, TH_g], F32, name="rk", tag="rk")
                    nc.scalar.activation(rk[:], z[:], Act.Exp, scale=-0.5)
                    # ppair free layout per quad: b(2) x tm(4) x h(16); each
                    # (q, b) slice is a contiguous 64-col run -> flat STTs
                    for q in range(nquad):
                        nc.vector.scalar_tensor_tensor(
                            ppair[:, q * P + b * 64 : q * P + b * 64 + 64],
                            ns_num[:, q * 64 : (q + 1) * 64],
                            0.0,
                            rk[:, q * 64 : (q + 1) * 64],
                            Alu.max,
                            Alu.mult,
                        )
                # transpose + drain per quad
                for q in range(nquad):
                    tp = pspool.tile([P, P], F32, name="tp", tag="tp")
                    nc.tensor.transpose(tp[:], ppair[:, q * P : (q + 1) * P], ident[:])
                    nc.scalar.copy(outacc[:, (gq + q) * P : (gq + q + 1) * P], tp[:])
                gq += nquad
                t0g += TGg

            for b in range(n_batch):
                eng = nc.sync
                for tm in range(4):
                    eng.dma_start(
                        out=out_r[b, :, :, tm, :],
                        in_=outacc[b * 64 + tm * H : b * 64 + (tm + 1) * H].rearrange(
                            "p (gq k) -> p gq k", k=P
                        ),
                    )
    return nc


_NC_CACHE: dict = {}


def _get_nc(n_batch, lk):
    key = (n_batch, lk)
    if key not in _NC_CACHE:
        _NC_CACHE[key] = build_nc(n_batch, lk)
    return _NC_CACHE[key]


def prep_inputs(query, key, mask, n_cores=N_CORES):
    """Shard + host-side input prep (layout & folding of scalars into qtilde)."""
    B, lk, dm = key.shape
    assert dm == DM
    nb = B // n_cores
    cdt_np = mybir.dt.np(BF16)

    q = query.reshape(B, H, DK).astype(np.float64)
    qn = np.sqrt((q * q).sum(-1))  # [B, H]
    qt = q / (qn[:, :, None] * float(lk))  # qtilde [B, H, DK]
    qb = np.ascontiguousarray(
        np.broadcast_to(qt.reshape(B, 1, DM), (B, P, DM))
    ).astype(cdt_np)

    ntiles = lk // P
    maskr = np.ascontiguousarray(
        mask.reshape(B, ntiles, P).transpose(0, 2, 1)
    ).astype(np.int32)
    ident = np.eye(P, dtype=np.float32)

    in_maps = []
    for c in range(n_cores):
        sl = slice(c * nb, (c + 1) * nb)
        in_maps.append(
            {
                "key": np.ascontiguousarray(key[sl]),
                "qb": qb[sl],
                "maskr": maskr[sl],
                "ident": ident,
            }
        )
    return in_maps


class _Runner:
    """Cached PJRT executable for one built Bass program.

    Mirrors bass2jax.run_bass_via_pjrt but jits ONCE, and feeds the
    global (unsharded) arrays directly: shard_map splits axis 0 across
    the 8 cores, which is exactly the per-core batch shard.
    """

    def __init__(self, nc, n_cores):
        import jax
        from jax.sharding import Mesh, PartitionSpec
        from jax.experimental.shard_map import shard_map
        from concourse import bass2jax as b2j

        b2j.install_neuronx_cc_hook()
        self.jax = jax
        self.n_cores = n_cores
        part_name = (
            nc.partition_id_tensor.name if nc.partition_id_tensor else None
        )
        in_names, out_names, out_avals, zero_outs = [], [], [], []
        for alloc in nc.m.functions[0].allocations:
            if not isinstance(alloc, mybir.MemoryLocationSet):
                continue
            name = alloc.memorylocations[0].name
            if alloc.kind == "ExternalInput":
                if name != part_name:
                    in_names.append(name)
            elif alloc.kind == "ExternalOutput":
                out_names.append(name)
                shape = tuple(alloc.tensor_shape)
                dtype = mybir.dt.np(alloc.dtype)
                out_avals.append(jax.core.ShapedArray(shape, dtype))
                zero_outs.append(np.zeros(shape, dtype))
        self.in_names, self.out_names = in_names, out_names
        self.out_avals, self.zero_outs = out_avals, zero_outs
        n_params, n_outs = len(in_names), len(out_names)

        bind_in_names = in_names + out_names
        if part_name is not None:
            bind_in_names = bind_in_names + [part_name]

        def _body(*args):
            operands = list(args)
            if part_name is not None:
                operands.append(b2j.partition_id_tensor())
            outs = b2j._bass_exec_p.bind(
                *operands,
                out_avals=tuple(out_avals),
                in_names=tuple(bind_in_names),
                out_names=tuple(out_names),
                lowering_input_output_aliases=(),
                sim_require_finite=True,
                sim_require_nnan=True,
                nc=nc,
            )
            return tuple(outs)

        devices = jax.devices()[:n_cores]
        self.mesh = Mesh(np.asarray(devices), ("core",))
        in_specs = (PartitionSpec("core"),) * (n_params + n_outs)
        out_specs = (PartitionSpec("core"),) * n_outs
        self.fn = jax.jit(
            shard_map(
                _body,
                mesh=self.mesh,
                in_specs=in_specs,
                out_specs=out_specs,
                check_rep=False,
            ),
            donate_argnums=tuple(range(n_params, n_params + n_outs)),
            keep_unused=True,
        )

    def global_args(self, global_ins: dict):
        args = [global_ins[name] for name in self.in_names]
        args += [
            np.zeros((self.n_cores * z.shape[0], *z.shape[1:]), z.dtype)
            for z in self.zero_outs
        ]
        return args

    def __call__(self, global_ins: dict):
        out_arrs = self.fn(*self.global_args(global_ins))
        return {
            name: np.asarray(out_arrs[i]) for i, name in enumerate(self.out_names)
        }


_RUNNER_CACHE: dict = {}


def _get_runner(n_batch, lk):
    key = (n_batch, lk)
    if key not in _RUNNER_CACHE:
        nc = _get_nc(n_batch, lk)
        if not nc.is_finalized():
            nc.finalize()
        _RUNNER_CACHE[key] = _Runner(nc, N_CORES)
    return _RUNNER_CACHE[key]


def global_inputs(query, key, mask):
    """Host prep producing the UNSHARDED arrays fed to shard_map (axis 0
    splits evenly across the 8 cores == batch sharding). Zero-copy for key."""
    B, lk, dm = key.shape
    assert dm == DM
    cdt_np = mybir.dt.np(BF16)

    q = query.reshape(B, H, DK).astype(np.float64)
    qn = np.sqrt((q * q).sum(-1))  # [B, H]
    qt = q / (qn[:, :, None] * float(lk))  # qtilde [B, H, DK]
    qb = np.ascontiguousarray(
        np.broadcast_to(qt.reshape(B, 1, DM), (B, P, DM))
    ).astype(cdt_np)

    ntiles = lk // P
    maskr = np.ascontiguousarray(
        mask.reshape(B, ntiles, P).transpose(0, 2, 1)
    ).astype(np.int32)

    ident = np.tile(np.eye(P, dtype=np.float32), (N_CORES, 1)).reshape(
        N_CORES * P, P
    )
    return {"key": np.ascontiguousarray(key), "qb": qb, "maskr": maskr,
            "ident": ident}


def kernel(query, key, mask):
    B, lk, _ = key.shape
    nb = B // N_CORES
    runner = _get_runner(nb, lk)
    gins = global_inputs(query, key, mask)
    out = runner(gins)["out"]  # [B, H, lk] concat over cores on axis 0
    return out.reshape(B, H, lk)


if __name__ == "__main__":
    # smoke test at reduced size
    rng = np.random.default_rng(0)
    B, lk = 16, 1024
    query = rng.standard_normal((B, 1, DM)).astype(np.float32)
    key = rng.standard_normal((B, lk, DM)).astype(np.float32)
    mask = rng.integers(0, 2, (B, lk)).astype(np.int32)
    out = kernel(query, key, mask)
    print("out", out.shape, out.dtype, float(np.abs(out).max()))


# revision 47
# speedup vs baseline: 1.1714x; 1.1714x over previous
"""Trainium2 Bass kernel for masked cosine-similarity attention scores.

Problem: nn_MultiHeadedAttention_2 (sparse_attention, memory-bound)
  query [16, 1, 1024] f32, key [16, 8192, 1024] f32, mask [16, 8192] int32
  out   [16, 16, 8192] f32 = relu(cos_sim_per_head(q, k) masked) / Lk

Math (per batch b, head h, key position l):
  num[h,l] = sum_d q[h,d] * k[l, h*64+d]
  kn[h,l]  = ||k[l, h*64:(h+1)*64]||
  p        = relu(num / (qn[h] * kn)) * mask[l] / Lk
           = relu(sum_d qtilde[h,d] * k[...]) * exp(-0.5*ln(kn^2) + lnm[h,l])
  where qtilde = q / (qn * Lk) is folded on the host (input prep) and
  lnm = 0 if mask else -1e30 (exp(...-1e30) == 0 -> exact masked zero).
  The reference's EPS=1e-8 guard on qn*kn is unreachable for randn inputs.

Sharding: data-parallel over batch B=16 -> 2 batches per core x 8 cores.

Engine layout (from NTFF hardware profiles; v1 of this kernel ran 434us/core,
this version ~315us/core):
  * All elementwise/reduce work is grouped 8 key-subtiles (1024 keys) per
    instruction: [128, 8192]-element ops, so the ~60ns DVE issue overhead and
    ~400ns small-op costs stop dominating (v1 had 876 small DVE ops).
  * ALL TT/reduce work runs on DVE: GPSIMD shares DVE's SBUF ports, so
    splitting folds onto it just splits the same bandwidth (measured: both
    engines slow ~1.4x when run concurrently). GPSIMD only runs the casting
    DMA descriptors. DVE tensor_tensor runs in 2x bf16 mode (4426ns per
    [128,8192] op, matching the (N/2+151)/0.96GHz formula).
  * num path: TT mult + fold chain 64->4 (2x mode) + one 1x tensor_reduce.
    tensor_reduce is 1x-only on TRN2, so folds do most of the reduction.
  * k^2 path: ACT Square (scalar engine, own port) + same DVE fold chain.
  * mask folded via lnm16 (per-(t,h) -1e30 offsets) + STT before a grouped
    Exp (ACT's bias operand is per-partition only, so exp(scale*ln+bias)
    cannot be grouped directly).
  * key streaming split across two DMA paths in parallel (alternating
    groups): SWDGE f32->bf16 cast (qPoolDynamic, ~250GB/s read) and HWDGE
    f32 (qSyncDynamicHW) + ACT cast, exceeding the single-queue rate.
  * outputs transposed head-major via TensorE (idle otherwise) in [128,128]
    blocks, drained by ACT, one strided HWDGE store per (batch, tm).

Self-contained: only imports the platform libs from /opt/trn_rl_repo.
"""

import sys

sys.path.insert(0, "/opt/trn_rl_repo")

import numpy as np

import concourse.bass as bass
import concourse.mybir as mybir
from concourse.tile import TileContext

# Keep the number of active DMA completion-sem lanes low: the kernel-tail
# Drain waits on every active proc's semaphore and walrus rejects
# instructions with too many sync waits.
import concourse.tile_sem_assignment as _tsa

_tsa.NUM_HWDGE_SEMS = 2
_tsa.NUM_SWDGE_GLOBAL_SEMS = 2

# The walrus build in this environment accepts at most ONE sync wait per
# instruction. Tile's scheduler can emit several (cross-engine RAW + WAR +
# DMA-lane waits). Splitting the extra waits into standalone EventSemaphore
# instructions on the same engine is semantically identical: the engine's
# sequencer executes them in order immediately before the instruction.
import orjson as _orjson


def _split_multi_waits(bir_bytes: bytes) -> bytes:
    m = _orjson.loads(bir_bytes)
    changed = False
    for fn in m.get("functions", []):
        for bb in fn.get("blocks", []):
            insts = bb.get("instructions")
            if not insts:
                continue
            out_list = []
            for inst in insts:
                si = inst.get("sync_info")
                waits = (si or {}).get("on_wait") or []
                if len(waits) > 1:
                    changed = True
                    for k, w in enumerate(waits[:-1]):
                        out_list.append(
                            {
                                "debug": inst.get("debug", 0),
                                "engine": inst["engine"],
                                "ins": [],
                                "name": f"{inst['name']}_wsplit{k}",
                                "opcode": "EventSemaphore",
                                "outs": [],
                                "sync_info": {"on_update": [], "on_wait": [w]},
                            }
                        )
                    si["on_wait"] = [waits[-1]]
                out_list.append(inst)
            bb["instructions"] = out_list
    return _orjson.dumps(m) if changed else bir_bytes


_orig_to_json_bytes = bass.Bass.to_json_bytes


def _patched_to_json_bytes(self, *a, **kw):
    return _split_multi_waits(_orig_to_json_bytes(self, *a, **kw))


bass.Bass.to_json_bytes = _patched_to_json_bytes

F32 = mybir.dt.float32
BF16 = mybir.dt.bfloat16
I32 = mybir.dt.int32
Alu = mybir.AluOpType
Act = mybir.ActivationFunctionType
AX = mybir.AxisListType

H = 16      # heads
DK = 64     # head dim
DM = 1024   # d_model
P = 128     # SBUF partitions
N_CORES = 8
TG = 8      # 128-key subtiles per group
# Each group's batch 0 loads via SWDGE f32->bf16 cast (qPoolDynamic) and
# batch 1 via HWDGE f32 (qSyncDynamicHW) + ACT cast, so BOTH DMA queues
# stream on every group instead of alternating between groups.


def self_fold_reduce(nc, pool, src, out, tag, d0=DK):
    """Segmented sum over d0-wide segments of src [P, S*d0] (bf16)
    -> out [P, S] f32, via DVE 2x-mode fold chain down to 4 + one 1x reduce.
    All instructions on DVE (GPSIMD shares DVE's SBUF ports; using it
    concurrently just splits the same bandwidth)."""
    S = src.free_size() // d0
    cur, d = src, d0
    while d > 4:
        nxt = pool.tile([P, S * d // 2], src.dtype, name=f"fold{tag}{d}",
                        tag=f"fold{d // 2}")
        c3 = cur.rearrange("p (s d) -> p s d", d=d)
        nc.vector.tensor_tensor(
            nxt.rearrange("p (s d) -> p s d", d=d // 2),
            c3[:, :, 0 : d // 2],
            c3[:, :, d // 2 : d],
            Alu.add,
        )
        cur, d = nxt, d // 2
    nc.vector.reduce_sum(out[:], cur.rearrange("p (s d) -> p s d", d=d),
                         axis=AX.X)


def build_nc(n_batch: int, lk: int) -> bass.Bass:
    """Per-core Bass program.

    Per-core DRAM I/O:
      key   [n_batch, lk, 1024] f32   (shard of the key tensor)
      qb    [n_batch, 128, 1024] bf16 (host-broadcast qtilde rows)
      maskr [n_batch, 128, lk/128] i32 (mask with l split as l = t*128 + p)
      out   [n_batch, 16, lk] f32
    """
    assert n_batch == 2, "kernel assumes a batch pair per core"
    cdt = BF16
    ntiles = lk // P
    ngroups = ntiles // TG

    nc = bass.Bass()
    key_in = nc.declare_dram_parameter("key", [n_batch, lk, DM], F32, isOutput=False)
    qb_in = nc.declare_dram_parameter("qb", [n_batch, P, DM], cdt, isOutput=False)
    mask_in = nc.declare_dram_parameter(
        "maskr", [n_batch, P, ntiles], I32, isOutput=False
    )
    ident_in = nc.declare_dram_parameter("ident", [P, P], F32, isOutput=False)
    out = nc.declare_dram_parameter("out", [n_batch, H, lk], F32, isOutput=True)
    # outacc partition = b*64 + tm*16 + h; key position l = (gq*4 + tm)*128 + k
    out_r = out.rearrange("b h (gq tm k) -> b h gq tm k", tm=4, k=P)

    with TileContext(nc) as tc:
        with (
            tc.tile_pool(name="const", bufs=1) as cpool,
            tc.tile_pool(name="kbig", bufs=3) as kpool,
            tc.tile_pool(name="stage", bufs=3) as stpool,
            tc.tile_pool(name="psboth", bufs=2) as pbpool,
            tc.tile_pool(name="halfp", bufs=1) as hpool,
            tc.tile_pool(name="small", bufs=4) as spool,
            tc.tile_pool(name="ppairp", bufs=3) as pppool,
            tc.tile_pool(name="outp", bufs=1) as opool,
            tc.tile_pool(name="psum", bufs=4, space="PSUM") as pspool,
        ):
            def load_group(t0g, TGg):
                kts = []
                for b in range(n_batch):
                    kt = kpool.tile([P, TGg * DM], cdt, name="kt", tag="kt")
                    src = key_in[b].rearrange("(t p) c -> p t c", p=P)[
                        :, t0g : t0g + TGg, :
                    ]
                    dst = kt.rearrange("p (t c) -> p t c", c=DM)
                    if b == 0:
                        # SWDGE cast f32->bf16
                        nc.gpsimd.dma_start(out=dst, in_=src)
                    else:
                        # HWDGE f32 (four quarters) + ACT cast
                        QTR = TGg * DM // 4
                        for hf in range(4):
                            st = stpool.tile([P, QTR], F32, name="st", tag="st")
                            nc.sync.dma_start(
                                out=st.rearrange("p (t c) -> p t c", c=DM),
                                in_=src[
                                    :, hf * (TGg // 4) : (hf + 1) * (TGg // 4), :
                                ],
                            )
                            nc.scalar.copy(kt[:, hf * QTR : (hf + 1) * QTR], st[:])
                    kts.append(kt)
                return kts

            sizes = [TG] * (ntiles // TG)
            assert sum(sizes) == ntiles

            # ---- constants / setup ----
            ident_r = cpool.tile([P, P], F32, name="ident_r")
            nc.gpsimd.dma_start(out=ident_r[:], in_=ident_in[:])
            ident = cpool.tile([P, P], F32, name="ident_s")
            nc.vector.tensor_copy(ident[:], ident_r[:])

            qbs, lnm16s = [], []
            for b in range(n_batch):
                qb_r = cpool.tile([P, DM], cdt, name=f"qbr{b}")
                nc.gpsimd.dma_start(out=qb_r[:], in_=qb_in[b])
                qb_s = cpool.tile([P, DM], cdt, name=f"qbs{b}")
                nc.scalar.copy(qb_s[:], qb_r[:])
                qbs.append(qb_s)

                maskt = cpool.tile([P, ntiles], I32, name=f"maskt{b}")
                nc.gpsimd.dma_start(out=maskt[:], in_=mask_in[b])
                maskf = cpool.tile([P, ntiles], F32, name=f"maskf{b}")
                nc.vector.tensor_copy(maskf[:], maskt[:])
                # expand [P, t] -> [P, t, 16] (repeat per head) in place,
                # then lnm16 = (m16 - 1) * 1e30  ->  {0 -> -1e30, 1 -> 0}
                lnm16 = cpool.tile([P, ntiles * H], F32, name=f"lnm16_{b}")
                m3 = lnm16.rearrange("p (t h) -> p t h", h=H)
                nc.vector.tensor_copy(
                    m3[:, :, 0:1], maskf.rearrange("p (t o) -> p t o", o=1)
                )
                w = 1
                while w < H:
                    nc.vector.tensor_copy(m3[:, :, w : 2 * w], m3[:, :, 0:w])
                    w *= 2
                # lnm2 = (m - 1) * -2e30 -> {0 -> +2e30, 1 -> 0}; Exp's
                # scale=-0.5 turns +2e30 into -1e30 -> exp -> exact 0
                nc.vector.tensor_scalar(
                    lnm16[:], lnm16[:], -1.0, -2.0e30, Alu.add, Alu.mult
                )
                lnm16s.append(lnm16)

            outacc = opool.tile([P, ngroups * 2 * P], F32, name="outacc")

            t0g = 0
            gq = 0  # global output quad index
            for gi, TGg in enumerate(sizes):
                kts = load_group(t0g, TGg)

                nquad = TGg // 4
                TH_g = TGg * H
                ppair = pppool.tile([P, nquad * P], F32, name="ppair", tag="ppair")
                for b in range(n_batch):
                    kt = kts[b]
                    # merged num + k^2 pipeline: products in the left half,
                    # squares in the right half of ONE double-buffered tile,
                    # so a single fold chain + reduce covers both segmented
                    # sums (4 fewer DVE issues/gb). Double-buffering (funded
                    # by the stride-0 qb broadcast replacing the 16KB qb8
                    # tiles) avoids the cross-group serialization that sank
                    # the single-buffered version of this merge.
                    ps = pbpool.tile([P, 2 * TGg * DM], cdt, name="ps",
                                     tag="ps")
                    qv = qbs[b][:]
                    qbc = bass.AP(
                        qv.tensor, qv.offset, [qv.ap[0], [0, TGg], qv.ap[1]]
                    )
                    nc.vector.tensor_tensor(
                        ps[:, 0 : TGg * DM].rearrange("p (t c) -> p t c", c=DM),
                        kt.rearrange("p (t c) -> p t c", c=DM),
                        qbc,
                        Alu.mult,
                    )
                    nc.scalar.activation(
                        ps[:, TGg * DM : 2 * TGg * DM], kt[:], Act.Square
                    )
                    nsb = spool.tile([P, 2 * TH_g], F32, name="nsb", tag="nsb")
                    self_fold_reduce(nc, hpool, ps, nsb, "b")
                    ns_num = nsb[:, 0:TH_g]
                    ns_sq = nsb[:, TH_g : 2 * TH_g]
                    # rk = exp(-0.5*(ln(s2) + lnm2)); the tiny add runs on
                    # GPSIMD (idle; [P,128] is too small to contend with DVE)
                    lns = spool.tile([P, TH_g], F32, name="lns", tag="lns")
                    nc.scalar.activation(lns[:], ns_sq[:], Act.Ln)
                    z = spool.tile([P, TH_g], F32, name="z", tag="z")
                    nc.gpsimd.tensor_tensor(
                        z[:],
                        lns[:],
                        lnm16s[b][:, t0g * H : (t0g + TGg) * H],
                        Alu.add,
                    )
                    rk = spool.tile([P, TH_g], F32, name="rk", tag="rk")
                    nc.scalar.activation(rk[:], z[:], Act.Exp, scale=-0.5)
                    # ppair free layout per quad: b(2) x tm(4) x h(16); each
                    # (q, b) slice is a contiguous 64-col run -> flat STTs
                    for q in range(nquad):
                        nc.vector.scalar_tensor_tensor(
                            ppair[:, q * P + b * 64 : q * P + b * 64 + 64],
                            ns_num[:, q * 64 : (q + 1) * 64],
                            0.0,
                            rk[:, q * 64 : (q + 1) * 64],
                            Alu.max,
                            Alu.mult,
                        )
                # transpose + drain per quad
                for q in range(nquad):
                    tp = pspool.tile([P, P], F32, name="tp", tag="tp")
                    nc.tensor.transpose(tp[:], ppair[:, q * P : (q + 1) * P], ident[:])
                    nc.scalar.copy(outacc[:, (gq + q) * P : (gq + q + 1) * P], tp[:])
                gq += nquad
                t0g += TGg

            for b in range(n_batch):
                eng = nc.sync
                for tm in range(4):
                    eng.dma_start(
                        out=out_r[b, :, :, tm, :],
                        in_=outacc[b * 64 + tm * H : b * 64 + (tm + 1) * H].rearrange(
                            "p (gq k) -> p gq k", k=P
                        ),
                    )
    return nc


_NC_CACHE: dict = {}


def _get_nc(n_batch, lk):
    key = (n_batch, lk)
    if key not in _NC_CACHE:
        _NC_CACHE[key] = build_nc(n_batch, lk)
    return _NC_CACHE[key]


def prep_inputs(query, key, mask, n_cores=N_CORES):
    """Shard + host-side input prep (layout & folding of scalars into qtilde)."""
    B, lk, dm = key.shape
    assert dm == DM
    nb = B // n_cores
    cdt_np = mybir.dt.np(BF16)

    q = query.reshape(B, H, DK).astype(np.float64)
    qn = np.sqrt((q * q).sum(-1))  # [B, H]
    qt = q / (qn[:, :, None] * float(lk))  # qtilde [B, H, DK]
    qb = np.ascontiguousarray(
        np.broadcast_to(qt.reshape(B, 1, DM), (B, P, DM))
    ).astype(cdt_np)

    ntiles = lk // P
    maskr = np.ascontiguousarray(
        mask.reshape(B, ntiles, P).transpose(0, 2, 1)
    ).astype(np.int32)
    ident = np.eye(P, dtype=np.float32)

    in_maps = []
    for c in range(n_cores):
        sl = slice(c * nb, (c + 1) * nb)
        in_maps.append(
            {
                "key": np.ascontiguousarray(key[sl]),
                "qb": qb[sl],
                "maskr": maskr[sl],
                "ident": ident,
            }
        )
    return in_maps


class _Runner:
    """Cached PJRT executable for one built Bass program.

    Mirrors bass2jax.run_bass_via_pjrt but jits ONCE, and feeds the
    global (unsharded) arrays directly: shard_map splits axis 0 across
    the 8 cores, which is exactly the per-core batch shard.
    """

    def __init__(self, nc, n_cores):
        import jax
        from jax.sharding import Mesh, PartitionSpec
        from jax.experimental.shard_map import shard_map
        from concourse import bass2jax as b2j

        b2j.install_neuronx_cc_hook()
        self.jax = jax
        self.n_cores = n_cores
        part_name = (
            nc.partition_id_tensor.name if nc.partition_id_tensor else None
        )
        in_names, out_names, out_avals, zero_outs = [], [], [], []
        for alloc in nc.m.functions[0].allocations:
            if not isinstance(alloc, mybir.MemoryLocationSet):
                continue
            name = alloc.memorylocations[0].name
            if alloc.kind == "ExternalInput":
                if name != part_name:
                    in_names.append(name)
            elif alloc.kind == "ExternalOutput":
                out_names.append(name)
                shape = tuple(alloc.tensor_shape)
                dtype = mybir.dt.np(alloc.dtype)
                out_avals.append(jax.core.ShapedArray(shape, dtype))
                zero_outs.append(np.zeros(shape, dtype))
        self.in_names, self.out_names = in_names, out_names
        self.out_avals, self.zero_outs = out_avals, zero_outs
        n_params, n_outs = len(in_names), len(out_names)

        bind_in_names = in_names + out_names
        if part_name is not None:
            bind_in_names = bind_in_names + [part_name]

        def _body(*args):
            operands = list(args)
            if part_name is not None:
                operands.append(b2j.partition_id_tensor())
            outs = b2j._bass_exec_p.bind(
                *operands,
                out_avals=tuple(out_avals),
                in_names=tuple(bind_in_names),
                out_names=tuple(out_names),
                lowering_input_output_aliases=(),
                sim_require_finite=True,
                sim_require_nnan=True,
                nc=nc,
            )
            return tuple(outs)

        devices = jax.devices()[:n_cores]
        self.mesh = Mesh(np.asarray(devices), ("core",))
        in_specs = (PartitionSpec("core"),) * (n_params + n_outs)
        out_specs = (PartitionSpec("core"),) * n_outs
        self.fn = jax.jit(
            shard_map(
                _body,
                mesh=self.mesh,
                in_specs=in_specs,
                out_specs=out_specs,
                check_rep=False,
            ),
            donate_argnums=tuple(range(n_params, n_params + n_outs)),
            keep_unused=True,
        )

    def global_args(self, global_ins: dict):
        args = [global_ins[name] for name in self.in_names]
        args += [
            np.zeros((self.n_cores * z.shape[0], *z.shape[1:]), z.dtype)
            for z in self.zero_outs
        ]
        return args

    def __call__(self, global_ins: dict):
        out_arrs = self.fn(*self.global_args(global_ins))
        return {
            name: np.asarray(out_arrs[i]) for i, name in enumerate(self.out_names)
        }


_RUNNER_CACHE: dict = {}


def _get_runner(n_batch, lk):
    key = (n_batch, lk)
    if key not in _RUNNER_CACHE:
        nc = _get_nc(n_batch, lk)
        if not nc.is_finalized():
            nc.finalize()
        _RUNNER_CACHE[key] = _Runner(nc, N_CORES)
    return _RUNNER_CACHE[key]


def global_inputs(query, key, mask):
    """Host prep producing the UNSHARDED arrays fed to shard_map (axis 0
    splits evenly across the 8 cores == batch sharding). Zero-copy for key."""
    B, lk, dm = key.shape
    assert dm == DM
    cdt_np = mybir.dt.np(BF16)

    q = query.reshape(B, H, DK).astype(np.float64)
    qn = np.sqrt((q * q).sum(-1))  # [B, H]
    qt = q / (qn[:, :, None] * float(lk))  # qtilde [B, H, DK]
    qb = np.ascontiguousarray(
        np.broadcast_to(qt.reshape(B, 1, DM), (B, P, DM))
    ).astype(cdt_np)

    ntiles = lk // P
    maskr = np.ascontiguousarray(
        mask.reshape(B, ntiles, P).transpose(0, 2, 1)
    ).astype(np.int32)

    ident = np.tile(np.eye(P, dtype=np.float32), (N_CORES, 1)).reshape(
        N_CORES * P, P
    )
    return {"key": np.ascontiguousarray(key), "qb": qb, "maskr": maskr,
            "ident": ident}


def kernel(query, key, mask):
    B, lk, _ = key.shape
    nb = B // N_CORES
    runner = _get_runner(nb, lk)
    gins = global_inputs(query, key, mask)
    out = runner(gins)["out"]  # [B, H, lk] concat over cores on axis 0
    return out.reshape(B, H, lk)


if __name__ == "__main__":
    # smoke test at reduced size
    rng = np.random.default_rng(0)
    B, lk = 16, 1024
    query = rng.standard_normal((B, 1, DM)).astype(np.float32)
    key = rng.standard_normal((B, lk, DM)).astype(np.float32)
    mask = rng.integers(0, 2, (B, lk)).astype(np.int32)
    out = kernel(query, key, mask)
    print("out", out.shape, out.dtype, float(np.abs(out).max()))
